# revision 18
# baseline (speedup 1.0000x reference)
"""Binarized 3x3 conv block on 8 Trainium2 NeuronCores — 1D-Winograd F(2,3).

v4 over the baseline:
- BN mean computed exactly on the host (conv-sum is linear in x: channel
  sums of x over 9 shifted windows, assembled from row/col/corner strips),
  so the device only reduces sum(y^2).
- A single end-of-kernel AllGather ([128,2] f32, one-shot mesh, ~5us
  floor) replaces the two ring AllReduces (~10us floor each, 25-38us
  observed, and the first one absorbs cross-core launch skew mid-kernel,
  which stalled the conv pipeline through engine-queue FIFO ordering).
- Fast path assumes gamma >= 0 (true for the shipped inputs; a general
  variant with the min-pool trick compiles on demand otherwise): maxpool
  commutes with the monotone BN apply, so min-pool tracking is dropped and
  the BN+ReLU apply is one fused op per image-chunk, split across the
  Vector (ch0) and Scalar (ch1) engines for tail throughput.
"""

import numpy as np
import ml_dtypes

_NCORES = 8
_B, _C, _H, _W = 32, 256, 56, 56
_BS = _B // _NCORES          # images per core
_PH, _PW = _H + 2, _W + 2    # padded input
_OH, _OW = _H // 2, _W // 2  # pooled output
_EPS = 1e-5
_NSTAT = float(_B * _H * _W)  # elements per channel in the BN stats
_BF16 = ml_dtypes.bfloat16

_CACHE: dict = {}


def _build(general: bool):
    import concourse.bacc as bacc
    import concourse.mybir as mybir
    import concourse.tile as tile

    f32 = mybir.dt.float32
    bf16 = mybir.dt.bfloat16
    AF = mybir.ActivationFunctionType
    AX = mybir.AxisListType
    OP = mybir.AluOpType

    nc = bacc.Bacc("TRN2", target_bir_lowering=False, debug=False,
                   num_devices=_NCORES)
    xp_d = nc.dram_tensor("xp", [_BS, _C, 2, _PH, _PW // 2], bf16,
                          kind="ExternalInput")
    w_d = nc.dram_tensor("wt", [2, 128, 12, _C], bf16, kind="ExternalInput")
    g_d = nc.dram_tensor("gm", [128, 2], f32, kind="ExternalInput")
    bt_d = nc.dram_tensor("bt", [128, 2], f32, kind="ExternalInput")
    mu_d = nc.dram_tensor("mu", [128, 2], f32, kind="ExternalInput")
    out_d = nc.dram_tensor("out", [_BS, _C, _OH, _OW], f32, kind="ExternalOutput")

    with tile.TileContext(nc) as tc:
        with (
            tc.tile_pool(name="persist", bufs=1) as keep,
            tc.tile_pool(name="xload", bufs=2) as xpool,
            tc.tile_pool(name="evict", bufs=3) as evp,
            tc.tile_pool(name="apply", bufs=4) as app,
            tc.tile_pool(name="acc", bufs=2, space="PSUM") as psp,
            tc.tile_pool(name="dram", bufs=1, space="DRAM") as dpool,
        ):
            # ---- weights on the scalar queue, x on sync/gpsimd queues ----
            w_sb = [keep.tile([128, 12, _C], bf16, tag=f"w{c}", name=f"w{c}")
                    for c in range(2)]
            for c in range(2):
                nc.scalar.dma_start(w_sb[c][:], w_d[c])
            gm_sb = keep.tile([128, 2], f32, tag="gm", name="gm")
            bt_sb = keep.tile([128, 2], f32, tag="bt", name="bt")
            mu_sb = keep.tile([128, 2], f32, tag="mu", name="mu")
            nc.scalar.dma_start(gm_sb[:], g_d[:])
            nc.scalar.dma_start(bt_sb[:], bt_d[:])
            nc.scalar.dma_start(mu_sb[:], mu_d[:])
            eps = keep.tile([128, 1], f32, tag="eps", name="eps")
            nc.gpsimd.memset(eps[:], _EPS)
            # prologue dummy Sqrt: pulls the sqrt-set ACT_TABLE_LOAD off the
            # post-collective tail into the idle kernel start
            warm = keep.tile([128, 1], f32, tag="warm", name="warm")
            nc.scalar.activation(warm[:], eps[:], AF.Sqrt, bias=0.0)

            # one sum-of-squares column per (img, rb)
            sqc = [keep.tile([128, 4 * _BS], f32, tag=f"sq{c}",
                             name=f"sq{c}") for c in range(2)]
            pmax = [[keep.tile([128, _OH, _OW], bf16, tag=f"pmax{i}_{c}",
                               name=f"pmax{i}_{c}") for c in range(2)]
                    for i in range(_BS)]
            if general:
                pmin = [[keep.tile([128, _OH, _OW], bf16, tag=f"pmin{i}_{c}",
                                   name=f"pmin{i}_{c}") for c in range(2)]
                        for i in range(_BS)]
            stats = keep.tile([128, 2], f32, tag="stats", name="stats")
            gat = keep.tile([128, 2, _NCORES], f32, tag="gat", name="gat")
            gsq = keep.tile([128, 2], f32, tag="gsq", name="gsq")

            # ---- width-axis input transforms, kept resident for both chunks
            # V0 = d0-d2, V1 = d1+d2, V2 = d2-d1, V3 = d1-d3 where
            # d0,d2 = adjacent even cols and d1,d3 = adjacent odd cols;
            # the host ships x as even/odd planes so every read is stride-1
            vt = [[None] * 2 for _ in range(_BS)]

            def emit_transforms(img):
                xs = []
                for cic in range(2):
                    vt[img][cic] = [keep.tile([128, _PH, _OW], bf16,
                                              tag=f"v{img}_{cic}_{l}",
                                              name=f"v{img}_{cic}_{l}")
                                    for l in range(4)]
                    xtile = xpool.tile([128, 2, _PH, _PW // 2], bf16,
                                       tag=f"x{cic}",
                                       name=f"x{img}_{cic}")
                    nc.sync.dma_start(xtile[:],
                                      xp_d[img, cic * 128:(cic + 1) * 128])
                    xs.append(xtile)
                # chunk-outer, l-major emission: half-height pieces keep the
                # vector-queue blocks short (evictions interleave without
                # stalling PSUM recycling) and let rb0's matmuls start after
                # the first two small ops instead of the full transform set
                for r0, r1 in ((0, 29), (29, _PH)):
                    for l in range(4):
                        for cic in range(2):
                            xe = xs[cic][:, 0, r0:r1]
                            xo = xs[cic][:, 1, r0:r1]
                            dst = vt[img][cic][l][:, r0:r1]
                            if l == 0:
                                nc.vector.tensor_sub(dst, xe[:, :, 0:_OW],
                                                     xe[:, :, 1:_OW + 1])
                            elif l == 1:
                                nc.vector.tensor_add(dst, xo[:, :, 0:_OW],
                                                     xe[:, :, 1:_OW + 1])
                            elif l == 2:
                                nc.vector.tensor_sub(dst, xe[:, :, 1:_OW + 1],
                                                     xo[:, :, 0:_OW])
                            else:
                                nc.vector.tensor_sub(dst, xo[:, :, 0:_OW],
                                                     xo[:, :, 1:_OW + 1])

            emit_transforms(0)
            emit_transforms(1)

            # ---- conv + fused eviction ----
            # 4 row-blocks of 14 output rows; the four Winograd products
            # live in one 4-bank PSUM tile (one 512-f32 bank per product)
            for ch in range(2):
                for img in range(_BS):
                    for rb in range(4):
                        ps = psp.tile([128, 4, 512], f32, tag="acc",
                                      name=f"acc{ch}_{img}_{rb}")
                        for l in range(4):
                            k = 0
                            for cic in range(2):
                                for kh in range(3):
                                    lhsT = w_sb[cic][:, l * 3 + kh,
                                                     ch * 128:(ch + 1) * 128]
                                    rhs = vt[img][cic][l][
                                        :, rb * 14 + kh: rb * 14 + kh + 14, :]
                                    nc.tensor.matmul(ps[:, l, 0:14 * _OW],
                                                     lhsT, rhs,
                                                     start=(k == 0),
                                                     stop=(k == 5))
                                    k += 1
                        # one ScalarE copy evicts all four products
                        mc = evp.tile([128, 4, 14, _OW], bf16, tag="mc",
                                      name=f"mc{ch}_{img}_{rb}")
                        nc.scalar.activation(
                            mc[:], ps[:, :, 0:14 * _OW], AF.Copy)
                        # even/odd output columns: yev=M0+M1+M2, yod=M1-M2-M3
                        yeo = evp.tile([128, 2, 14, _OW], bf16, tag="yeo",
                                       name=f"yeo{ch}_{img}_{rb}")
                        t01 = evp.tile([128, 14, _OW], bf16, tag="t01",
                                       name=f"t01_{ch}_{img}_{rb}")
                        nc.vector.tensor_add(t01[:], mc[:, 0], mc[:, 1])
                        t12 = evp.tile([128, 14, _OW], bf16, tag="t12",
                                       name=f"t12_{ch}_{img}_{rb}")
                        nc.vector.tensor_sub(t12[:], mc[:, 1], mc[:, 2])
                        col = img * 4 + rb
                        nc.vector.tensor_add(yeo[:, 0], t01[:], mc[:, 2])
                        nc.vector.tensor_sub(yeo[:, 1], t12[:], mc[:, 3])
                        sq1 = evp.tile([128, 2, 14, _OW], bf16, tag="sq1",
                                       name=f"sq1_{ch}_{img}_{rb}")
                        nc.scalar.activation(sq1[:], yeo[:], AF.Square,
                                             accum_out=sqc[ch][:, col:col + 1])
                        # 2x2 pools: even/odd col split == pool col pairing
                        t1 = evp.tile([128, 7, _OW], bf16, tag="t1",
                                      name=f"t1_{ch}_{img}_{rb}")
                        t2 = evp.tile([128, 7, _OW], bf16, tag="t2",
                                      name=f"t2_{ch}_{img}_{rb}")
                        nc.vector.tensor_max(t1[:], yeo[:, 0, 0:14:2, :],
                                             yeo[:, 0, 1:14:2, :])
                        nc.vector.tensor_max(t2[:], yeo[:, 1, 0:14:2, :],
                                             yeo[:, 1, 1:14:2, :])
                        nc.vector.tensor_max(
                            pmax[img][ch][:, rb * 7:(rb + 1) * 7, :],
                            t1[:], t2[:])
                        if general:
                            t3 = evp.tile([128, 7, _OW], bf16, tag="t3",
                                          name=f"t3_{ch}_{img}_{rb}")
                            t4 = evp.tile([128, 7, _OW], bf16, tag="t4",
                                          name=f"t4_{ch}_{img}_{rb}")
                            nc.vector.tensor_tensor(
                                t3[:], yeo[:, 0, 0:14:2, :],
                                yeo[:, 0, 1:14:2, :], op=OP.min)
                            nc.vector.tensor_tensor(
                                t4[:], yeo[:, 1, 0:14:2, :],
                                yeo[:, 1, 1:14:2, :], op=OP.min)
                            nc.vector.tensor_tensor(
                                pmin[img][ch][:, rb * 7:(rb + 1) * 7, :],
                                t3[:], t4[:], op=OP.min)
                        if ch == 0 and img == 0 and rb in (0, 1):
                            emit_transforms(2 if rb == 0 else 3)

                # chunk's local sum(y^2) column
                nc.vector.reduce_sum(stats[:, ch:ch + 1], sqc[ch][:],
                                     axis=AX.X)

            # ---- one AllGather for both chunks + local 8-way reduce ----
            cc_in = dpool.tile([128, 2], f32, tag="ccin", name="ccin")
            cc_out = dpool.tile([_NCORES, 128, 2], f32, tag="ccout",
                                name="ccout")
            nc.scalar.dma_start(cc_in[:], stats[:])
            nc.gpsimd.collective_compute(
                "AllGather", OP.bypass,
                replica_groups=[list(range(_NCORES))],
                ins=[cc_in.opt()], outs=[cc_out.opt()])
            nc.sync.dma_start(gat[:], cc_out[:].transpose([1, 2, 0]))
            nc.vector.tensor_reduce(gsq[:], gat[:], op=OP.add, axis=AX.X)

            # ---- scale/bias for both chunks in one [128,2]-wide pass ----
            m2 = keep.tile([128, 2], f32, tag="m2", name="m2")
            var = keep.tile([128, 2], f32, tag="var", name="var")
            sd = keep.tile([128, 2], f32, tag="sd", name="sd")
            inv = keep.tile([128, 2], f32, tag="inv", name="inv")
            s = keep.tile([128, 2], f32, tag="s", name="s")
            ms_ = keep.tile([128, 2], f32, tag="ms", name="ms")
            bb = keep.tile([128, 2], f32, tag="bb", name="bb")
            nc.vector.tensor_mul(m2[:], mu_sb[:], mu_sb[:])
            nc.vector.tensor_scalar(var[:], gsq[:], 1.0 / _NSTAT, None,
                                    op0=OP.mult)
            nc.vector.tensor_sub(var[:], var[:], m2[:])
            nc.scalar.activation(sd[:], var[:], AF.Sqrt, bias=eps[:])
            nc.vector.reciprocal(inv[:], sd[:])
            nc.vector.tensor_mul(s[:], gm_sb[:], inv[:])
            nc.vector.tensor_mul(ms_[:], mu_sb[:], s[:])
            nc.vector.tensor_sub(bb[:], bt_sb[:], ms_[:])

            # ---- BN/ReLU apply + store; ch0 on VectorE, ch1 on ScalarE ----
            for img in range(_BS):
                for ch in range(2):
                    res = app.tile([128, _OH, _OW], f32, tag=f"res{ch}",
                                   name=f"res{ch}_{img}")
                    if general:
                        u = app.tile([128, _OH, _OW], bf16, tag=f"u{ch}",
                                     name=f"u{ch}_{img}")
                        v = app.tile([128, _OH, _OW], bf16, tag=f"v{ch}",
                                     name=f"v{ch}_{img}")
                        m = app.tile([128, _OH, _OW], bf16, tag=f"m{ch}",
                                     name=f"m{ch}_{img}")
                        nc.vector.tensor_scalar_mul(u[:], pmax[img][ch][:],
                                                    s[:, ch:ch + 1])
                        nc.vector.tensor_scalar_mul(v[:], pmin[img][ch][:],
                                                    s[:, ch:ch + 1])
                        nc.vector.tensor_max(m[:], u[:], v[:])
                        nc.scalar.activation(res[:], m[:], AF.Relu,
                                             bias=bb[:, ch:ch + 1])
                    elif ch == 0:
                        nc.vector.tensor_scalar(res[:], pmax[img][0][:],
                                                s[:, 0:1], bb[:, 0:1],
                                                op0=OP.mult, op1=OP.add)
                        nc.vector.tensor_scalar_max(res[:], res[:], 0.0)
                    else:
                        nc.scalar.activation(res[:], pmax[img][1][:],
                                             AF.Relu, bias=bb[:, 1:2],
                                             scale=s[:, 1:2])
                    eng = nc.sync if ch == 0 else nc.scalar
                    eng.dma_start(out_d[img, ch * 128:(ch + 1) * 128], res[:])

    nc.compile()
    return nc


def _host_mean(x64, g):
    """Exact per-channel mean of conv(x, sign(W)) over (batch, H, W):
    the conv-sum is linear in x, so it reduces to channel sums of x over
    the 9 (kh, kw)-shifted valid windows, assembled from strip sums."""
    B, C, H, W = x64.shape
    T = x64.sum((0, 2, 3))
    R = x64.sum((0, 3))
    Cc = x64.sum((0, 2))
    corner = {(hh, ww): x64[:, :, hh, ww].sum(0)
              for hh in (0, H - 1) for ww in (0, W - 1)}

    def S(dh, dw):
        sv = T.copy()
        er = [] if dh == 0 else ([H - 1] if dh < 0 else [0])
        ec = [] if dw == 0 else ([W - 1] if dw < 0 else [0])
        for r in er:
            sv = sv - R[:, r]
        for cl in ec:
            sv = sv - Cc[:, cl]
        for r in er:
            for cl in ec:
                sv = sv + corner[(r, cl)]
        return sv

    Sm = np.stack([np.stack([S(dh, dw) for dw in (-1, 0, 1)])
                   for dh in (-1, 0, 1)])          # [3(kh), 3(kw), C]
    return np.einsum('oihw,hwi->o', g, Sm) / (B * H * W)


def _prep_inputs(x, W, gamma, beta):
    x = np.asarray(x, dtype=np.float32)
    W = np.asarray(W, dtype=np.float32)
    gamma = np.asarray(gamma, dtype=np.float32)
    beta = np.asarray(beta, dtype=np.float32)

    # Winograd F(2,3) width-axis weight transform of the binarized weights:
    # U0 = g0, U1 = (g0+g1+g2)/2, U2 = (g0-g1+g2)/2, U3 = g2.
    # All values are exact in bf16.
    g = np.sign(W)                                     # [co, ci, kh, kw]
    u4 = np.stack([
        g[..., 0],
        (g[..., 0] + g[..., 1] + g[..., 2]) * 0.5,
        (g[..., 0] - g[..., 1] + g[..., 2]) * 0.5,
        g[..., 2],
    ], axis=0)                                         # [4l, co, ci, 3kh]
    wt = u4.transpose(2, 0, 3, 1).reshape(2, 128, 12, _C)
    wt = np.ascontiguousarray(wt).astype(_BF16)

    mu = _host_mean(x.astype(np.float64), g).astype(np.float32)
    mu = np.ascontiguousarray(mu.reshape(2, 128).T)          # [128, 2]

    xp = np.zeros((_B, _C, _PH, _PW), dtype=_BF16)
    xp[:, :, 1:_H + 1, 1:_W + 1] = x.astype(_BF16)
    # even/odd column planes -> all device-side transforms are stride-1
    xp = np.ascontiguousarray(
        np.stack([xp[..., 0::2], xp[..., 1::2]], axis=2))

    gm = np.ascontiguousarray(gamma.reshape(2, 128).T)       # [128, 2]
    bt = np.ascontiguousarray(beta.reshape(2, 128).T)

    in_maps = []
    for core in range(_NCORES):
        in_maps.append({
            "xp": np.ascontiguousarray(xp[core * _BS:(core + 1) * _BS]),
            "wt": wt,
            "gm": gm,
            "bt": bt,
            "mu": mu,
        })
    return in_maps


def _run(x, W, gamma, beta, trace=False):
    from concourse.bass_utils import run_bass_kernel_spmd

    general = bool(np.asarray(gamma).min() < 0)
    key = f"nc_{general}"
    if key not in _CACHE:
        _CACHE[key] = _build(general)
    nc = _CACHE[key]
    in_maps = _prep_inputs(x, W, gamma, beta)
    res = run_bass_kernel_spmd(nc, in_maps, core_ids=list(range(_NCORES)),
                               trace=trace)
    out = np.concatenate([res.results[c]["out"] for c in range(_NCORES)], axis=0)
    return np.ascontiguousarray(out.astype(np.float32)), res


def kernel(x, W, gamma, beta):
    out, _ = _run(x, W, gamma, beta, trace=False)
    return out


# revision 24
# speedup vs baseline: 1.0025x; 1.0025x over previous
"""Binarized 3x3 conv block on 8 Trainium2 NeuronCores — 1D-Winograd F(2,3).

v4 over the baseline:
- BN mean computed exactly on the host (conv-sum is linear in x: channel
  sums of x over 9 shifted windows, assembled from row/col/corner strips),
  so the device only reduces sum(y^2).
- A single end-of-kernel AllGather ([128,2] f32, one-shot mesh, ~5us
  floor) replaces the two ring AllReduces (~10us floor each, 25-38us
  observed, and the first one absorbs cross-core launch skew mid-kernel,
  which stalled the conv pipeline through engine-queue FIFO ordering).
- Fast path assumes gamma >= 0 (true for the shipped inputs; a general
  variant with the min-pool trick compiles on demand otherwise): maxpool
  commutes with the monotone BN apply, so min-pool tracking is dropped and
  the BN+ReLU apply is one fused op per image-chunk, split across the
  Vector (ch0) and Scalar (ch1) engines for tail throughput.
"""

import numpy as np
import ml_dtypes

_NCORES = 8
_B, _C, _H, _W = 32, 256, 56, 56
_BS = _B // _NCORES          # images per core
_PH, _PW = _H + 2, _W + 2    # padded input
_OH, _OW = _H // 2, _W // 2  # pooled output
_EPS = 1e-5
_NSTAT = float(_B * _H * _W)  # elements per channel in the BN stats
_BF16 = ml_dtypes.bfloat16

_CACHE: dict = {}


def _build(general: bool):
    import concourse.bacc as bacc
    import concourse.mybir as mybir
    import concourse.tile as tile

    f32 = mybir.dt.float32
    bf16 = mybir.dt.bfloat16
    AF = mybir.ActivationFunctionType
    AX = mybir.AxisListType
    OP = mybir.AluOpType

    nc = bacc.Bacc("TRN2", target_bir_lowering=False, debug=False,
                   num_devices=_NCORES)
    xp_d = nc.dram_tensor("xp", [_BS, _C, 2, _PH, _PW // 2], bf16,
                          kind="ExternalInput")
    w_d = nc.dram_tensor("wt", [2, 128, 12, _C], bf16, kind="ExternalInput")
    g_d = nc.dram_tensor("gm", [128, 2], f32, kind="ExternalInput")
    bt_d = nc.dram_tensor("bt", [128, 2], f32, kind="ExternalInput")
    mu_d = nc.dram_tensor("mu", [128, 2], f32, kind="ExternalInput")
    out_d = nc.dram_tensor("out", [_BS, _C, _OH, _OW], f32, kind="ExternalOutput")

    with tile.TileContext(nc) as tc:
        with (
            tc.tile_pool(name="persist", bufs=1) as keep,
            tc.tile_pool(name="xload", bufs=2) as xpool,
            tc.tile_pool(name="evict", bufs=3) as evp,
            tc.tile_pool(name="apply", bufs=4) as app,
            tc.tile_pool(name="acc", bufs=2, space="PSUM") as psp,
            tc.tile_pool(name="dram", bufs=1, space="DRAM") as dpool,
        ):
            w_sb = [keep.tile([128, 12, _C], bf16, tag=f"w{c}", name=f"w{c}")
                    for c in range(2)]
            gm_sb = keep.tile([128, 2], f32, tag="gm", name="gm")
            bt_sb = keep.tile([128, 2], f32, tag="bt", name="bt")
            mu_sb = keep.tile([128, 2], f32, tag="mu", name="mu")
            eps = keep.tile([128, 1], f32, tag="eps", name="eps")
            nc.gpsimd.memset(eps[:], _EPS)
            warm = keep.tile([128, 1], f32, tag="warm", name="warm")

            # one sum-of-squares column per (img, rb)
            sqc = [keep.tile([128, 4 * _BS], f32, tag=f"sq{c}",
                             name=f"sq{c}") for c in range(2)]
            pmax = [[keep.tile([128, _OH, _OW], bf16, tag=f"pmax{i}_{c}",
                               name=f"pmax{i}_{c}") for c in range(2)]
                    for i in range(_BS)]
            if general:
                pmin = [[keep.tile([128, _OH, _OW], bf16, tag=f"pmin{i}_{c}",
                                   name=f"pmin{i}_{c}") for c in range(2)]
                        for i in range(_BS)]
            stats = keep.tile([128, 2], f32, tag="stats", name="stats")
            gat = keep.tile([128, 2, _NCORES], f32, tag="gat", name="gat")
            gsq = keep.tile([128, 2], f32, tag="gsq", name="gsq")

            # ---- width-axis input transforms, kept resident for both chunks
            # V0 = d0-d2, V1 = d1+d2, V2 = d2-d1, V3 = d1-d3 where
            # d0,d2 = adjacent even cols and d1,d3 = adjacent odd cols;
            # the host ships x as even/odd planes so every read is stride-1
            vt = [[None] * 2 for _ in range(_BS)]

            def emit_transforms(img):
                xs = []
                for cic in range(2):
                    vt[img][cic] = [keep.tile([128, _PH, _OW], bf16,
                                              tag=f"v{img}_{cic}_{l}",
                                              name=f"v{img}_{cic}_{l}")
                                    for l in range(4)]
                    xtile = xpool.tile([128, 2, _PH, _PW // 2], bf16,
                                       tag=f"x{cic}",
                                       name=f"x{img}_{cic}")
                    if cic == 0:
                        eng = nc.sync
                    else:
                        eng = nc.scalar if img == 0 else nc.gpsimd
                    eng.dma_start(xtile[:],
                                  xp_d[img, cic * 128:(cic + 1) * 128])
                    xs.append(xtile)
                # chunk-outer, l-major emission: half-height pieces keep the
                # vector-queue blocks short (evictions interleave without
                # stalling PSUM recycling) and let rb0's matmuls start after
                # the first two small ops instead of the full transform set
                for r0, r1 in ((0, 29), (29, _PH)):
                    for l in range(4):
                        for cic in range(2):
                            xe = xs[cic][:, 0, r0:r1]
                            xo = xs[cic][:, 1, r0:r1]
                            dst = vt[img][cic][l][:, r0:r1]
                            if l == 0:
                                nc.vector.tensor_sub(dst, xe[:, :, 0:_OW],
                                                     xe[:, :, 1:_OW + 1])
                            elif l == 1:
                                nc.vector.tensor_add(dst, xo[:, :, 0:_OW],
                                                     xe[:, :, 1:_OW + 1])
                            elif l == 2:
                                nc.vector.tensor_sub(dst, xe[:, :, 1:_OW + 1],
                                                     xo[:, :, 0:_OW])
                            else:
                                nc.vector.tensor_sub(dst, xo[:, :, 0:_OW],
                                                     xo[:, :, 1:_OW + 1])

            # img0's x loads lead the sync/scalar queues so the first rb's
            # transforms (and matmuls) are not gated on the weight transfers
            emit_transforms(0)
            emit_transforms(1)
            for c in range(2):
                nc.scalar.dma_start(w_sb[c][:], w_d[c])
            nc.scalar.dma_start(gm_sb[:], g_d[:])
            nc.scalar.dma_start(bt_sb[:], bt_d[:])
            nc.scalar.dma_start(mu_sb[:], mu_d[:])
            # prologue dummy Sqrt: pulls the sqrt-set ACT_TABLE_LOAD off the
            # post-collective tail into the idle kernel start
            nc.scalar.activation(warm[:], eps[:], AF.Sqrt, bias=0.0)

            # ---- conv + fused eviction ----
            # 4 row-blocks of 14 output rows; the four Winograd products
            # live in one 4-bank PSUM tile (one 512-f32 bank per product)
            for ch in range(2):
                for img in range(_BS):
                    for rb in range(4):
                        ps = psp.tile([128, 4, 512], f32, tag="acc",
                                      name=f"acc{ch}_{img}_{rb}")
                        for l in range(4):
                            k = 0
                            for cic in range(2):
                                for kh in range(3):
                                    lhsT = w_sb[cic][:, l * 3 + kh,
                                                     ch * 128:(ch + 1) * 128]
                                    rhs = vt[img][cic][l][
                                        :, rb * 14 + kh: rb * 14 + kh + 14, :]
                                    nc.tensor.matmul(ps[:, l, 0:14 * _OW],
                                                     lhsT, rhs,
                                                     start=(k == 0),
                                                     stop=(k == 5))
                                    k += 1
                        # one ScalarE copy evicts all four products
                        mc = evp.tile([128, 4, 14, _OW], bf16, tag="mc",
                                      name=f"mc{ch}_{img}_{rb}")
                        nc.scalar.activation(
                            mc[:], ps[:, :, 0:14 * _OW], AF.Copy)
                        # even/odd output columns: yev=M0+M1+M2, yod=M1-M2-M3
                        yeo = evp.tile([128, 2, 14, _OW], bf16, tag="yeo",
                                       name=f"yeo{ch}_{img}_{rb}")
                        t01 = evp.tile([128, 14, _OW], bf16, tag="t01",
                                       name=f"t01_{ch}_{img}_{rb}")
                        nc.vector.tensor_add(t01[:], mc[:, 0], mc[:, 1])
                        t12 = evp.tile([128, 14, _OW], bf16, tag="t12",
                                       name=f"t12_{ch}_{img}_{rb}")
                        nc.vector.tensor_sub(t12[:], mc[:, 1], mc[:, 2])
                        col = img * 4 + rb
                        nc.vector.tensor_add(yeo[:, 0], t01[:], mc[:, 2])
                        nc.vector.tensor_sub(yeo[:, 1], t12[:], mc[:, 3])
                        sq1 = evp.tile([128, 2, 14, _OW], bf16, tag="sq1",
                                       name=f"sq1_{ch}_{img}_{rb}")
                        nc.scalar.activation(sq1[:], yeo[:], AF.Square,
                                             accum_out=sqc[ch][:, col:col + 1])
                        # 2x2 pools: even/odd col split == pool col pairing
                        t1 = evp.tile([128, 7, _OW], bf16, tag="t1",
                                      name=f"t1_{ch}_{img}_{rb}")
                        t2 = evp.tile([128, 7, _OW], bf16, tag="t2",
                                      name=f"t2_{ch}_{img}_{rb}")
                        nc.vector.tensor_max(t1[:], yeo[:, 0, 0:14:2, :],
                                             yeo[:, 0, 1:14:2, :])
                        nc.vector.tensor_max(t2[:], yeo[:, 1, 0:14:2, :],
                                             yeo[:, 1, 1:14:2, :])
                        nc.vector.tensor_max(
                            pmax[img][ch][:, rb * 7:(rb + 1) * 7, :],
                            t1[:], t2[:])
                        if general:
                            t3 = evp.tile([128, 7, _OW], bf16, tag="t3",
                                          name=f"t3_{ch}_{img}_{rb}")
                            t4 = evp.tile([128, 7, _OW], bf16, tag="t4",
                                          name=f"t4_{ch}_{img}_{rb}")
                            nc.vector.tensor_tensor(
                                t3[:], yeo[:, 0, 0:14:2, :],
                                yeo[:, 0, 1:14:2, :], op=OP.min)
                            nc.vector.tensor_tensor(
                                t4[:], yeo[:, 1, 0:14:2, :],
                                yeo[:, 1, 1:14:2, :], op=OP.min)
                            nc.vector.tensor_tensor(
                                pmin[img][ch][:, rb * 7:(rb + 1) * 7, :],
                                t3[:], t4[:], op=OP.min)
                        if ch == 0 and img == 0 and rb in (0, 1):
                            emit_transforms(2 if rb == 0 else 3)

                # chunk's local sum(y^2) column
                nc.vector.reduce_sum(stats[:, ch:ch + 1], sqc[ch][:],
                                     axis=AX.X)

            # ---- one AllGather for both chunks + local 8-way reduce ----
            cc_in = dpool.tile([128, 2], f32, tag="ccin", name="ccin")
            cc_out = dpool.tile([_NCORES, 128, 2], f32, tag="ccout",
                                name="ccout")
            nc.scalar.dma_start(cc_in[:], stats[:])
            nc.gpsimd.collective_compute(
                "AllGather", OP.bypass,
                replica_groups=[list(range(_NCORES))],
                ins=[cc_in.opt()], outs=[cc_out.opt()])
            nc.sync.dma_start(gat[:], cc_out[:].transpose([1, 2, 0]))
            nc.vector.tensor_reduce(gsq[:], gat[:], op=OP.add, axis=AX.X)

            # ---- scale/bias for both chunks in one [128,2]-wide pass ----
            m2 = keep.tile([128, 2], f32, tag="m2", name="m2")
            var = keep.tile([128, 2], f32, tag="var", name="var")
            sd = keep.tile([128, 2], f32, tag="sd", name="sd")
            inv = keep.tile([128, 2], f32, tag="inv", name="inv")
            s = keep.tile([128, 2], f32, tag="s", name="s")
            ms_ = keep.tile([128, 2], f32, tag="ms", name="ms")
            bb = keep.tile([128, 2], f32, tag="bb", name="bb")
            nc.vector.tensor_mul(m2[:], mu_sb[:], mu_sb[:])
            nc.vector.tensor_scalar(var[:], gsq[:], 1.0 / _NSTAT, None,
                                    op0=OP.mult)
            nc.vector.tensor_sub(var[:], var[:], m2[:])
            nc.scalar.activation(sd[:], var[:], AF.Sqrt, bias=eps[:])
            nc.vector.reciprocal(inv[:], sd[:])
            nc.vector.tensor_mul(s[:], gm_sb[:], inv[:])
            nc.vector.tensor_mul(ms_[:], mu_sb[:], s[:])
            nc.vector.tensor_sub(bb[:], bt_sb[:], ms_[:])

            # ---- BN/ReLU apply + store; ch0 on VectorE, ch1 on ScalarE ----
            for img in range(_BS):
                for ch in range(2):
                    res = app.tile([128, _OH, _OW], f32, tag=f"res{ch}",
                                   name=f"res{ch}_{img}")
                    if general:
                        u = app.tile([128, _OH, _OW], bf16, tag=f"u{ch}",
                                     name=f"u{ch}_{img}")
                        v = app.tile([128, _OH, _OW], bf16, tag=f"v{ch}",
                                     name=f"v{ch}_{img}")
                        m = app.tile([128, _OH, _OW], bf16, tag=f"m{ch}",
                                     name=f"m{ch}_{img}")
                        nc.vector.tensor_scalar_mul(u[:], pmax[img][ch][:],
                                                    s[:, ch:ch + 1])
                        nc.vector.tensor_scalar_mul(v[:], pmin[img][ch][:],
                                                    s[:, ch:ch + 1])
                        nc.vector.tensor_max(m[:], u[:], v[:])
                        nc.scalar.activation(res[:], m[:], AF.Relu,
                                             bias=bb[:, ch:ch + 1])
                    elif ch == 0:
                        nc.vector.tensor_scalar(res[:], pmax[img][0][:],
                                                s[:, 0:1], bb[:, 0:1],
                                                op0=OP.mult, op1=OP.add)
                        nc.vector.tensor_scalar_max(res[:], res[:], 0.0)
                    else:
                        nc.scalar.activation(res[:], pmax[img][1][:],
                                             AF.Relu, bias=bb[:, 1:2],
                                             scale=s[:, 1:2])
                    eng = nc.sync if ch == 0 else nc.gpsimd
                    eng.dma_start(out_d[img, ch * 128:(ch + 1) * 128], res[:])

    nc.compile()
    return nc


def _host_mean(x64, g):
    """Exact per-channel mean of conv(x, sign(W)) over (batch, H, W):
    the conv-sum is linear in x, so it reduces to channel sums of x over
    the 9 (kh, kw)-shifted valid windows, assembled from strip sums."""
    B, C, H, W = x64.shape
    T = x64.sum((0, 2, 3))
    R = x64.sum((0, 3))
    Cc = x64.sum((0, 2))
    corner = {(hh, ww): x64[:, :, hh, ww].sum(0)
              for hh in (0, H - 1) for ww in (0, W - 1)}

    def S(dh, dw):
        sv = T.copy()
        er = [] if dh == 0 else ([H - 1] if dh < 0 else [0])
        ec = [] if dw == 0 else ([W - 1] if dw < 0 else [0])
        for r in er:
            sv = sv - R[:, r]
        for cl in ec:
            sv = sv - Cc[:, cl]
        for r in er:
            for cl in ec:
                sv = sv + corner[(r, cl)]
        return sv

    Sm = np.stack([np.stack([S(dh, dw) for dw in (-1, 0, 1)])
                   for dh in (-1, 0, 1)])          # [3(kh), 3(kw), C]
    return np.einsum('oihw,hwi->o', g, Sm) / (B * H * W)


def _prep_inputs(x, W, gamma, beta):
    x = np.asarray(x, dtype=np.float32)
    W = np.asarray(W, dtype=np.float32)
    gamma = np.asarray(gamma, dtype=np.float32)
    beta = np.asarray(beta, dtype=np.float32)

    # Winograd F(2,3) width-axis weight transform of the binarized weights:
    # U0 = g0, U1 = (g0+g1+g2)/2, U2 = (g0-g1+g2)/2, U3 = g2.
    # All values are exact in bf16.
    g = np.sign(W)                                     # [co, ci, kh, kw]
    u4 = np.stack([
        g[..., 0],
        (g[..., 0] + g[..., 1] + g[..., 2]) * 0.5,
        (g[..., 0] - g[..., 1] + g[..., 2]) * 0.5,
        g[..., 2],
    ], axis=0)                                         # [4l, co, ci, 3kh]
    wt = u4.transpose(2, 0, 3, 1).reshape(2, 128, 12, _C)
    wt = np.ascontiguousarray(wt).astype(_BF16)

    mu = _host_mean(x.astype(np.float64), g).astype(np.float32)
    mu = np.ascontiguousarray(mu.reshape(2, 128).T)          # [128, 2]

    xp = np.zeros((_B, _C, _PH, _PW), dtype=_BF16)
    xp[:, :, 1:_H + 1, 1:_W + 1] = x.astype(_BF16)
    # even/odd column planes -> all device-side transforms are stride-1
    xp = np.ascontiguousarray(
        np.stack([xp[..., 0::2], xp[..., 1::2]], axis=2))

    gm = np.ascontiguousarray(gamma.reshape(2, 128).T)       # [128, 2]
    bt = np.ascontiguousarray(beta.reshape(2, 128).T)

    in_maps = []
    for core in range(_NCORES):
        in_maps.append({
            "xp": np.ascontiguousarray(xp[core * _BS:(core + 1) * _BS]),
            "wt": wt,
            "gm": gm,
            "bt": bt,
            "mu": mu,
        })
    return in_maps


def _run(x, W, gamma, beta, trace=False):
    from concourse.bass_utils import run_bass_kernel_spmd

    general = bool(np.asarray(gamma).min() < 0)
    key = f"nc_{general}"
    if key not in _CACHE:
        _CACHE[key] = _build(general)
    nc = _CACHE[key]
    in_maps = _prep_inputs(x, W, gamma, beta)
    res = run_bass_kernel_spmd(nc, in_maps, core_ids=list(range(_NCORES)),
                               trace=trace)
    out = np.concatenate([res.results[c]["out"] for c in range(_NCORES)], axis=0)
    return np.ascontiguousarray(out.astype(np.float32)), res


def kernel(x, W, gamma, beta):
    out, _ = _run(x, W, gamma, beta, trace=False)
    return out
